# revision 3
# baseline (speedup 1.0000x reference)
"""Trainium2 Bass kernel for CommittorNetBP (pairwise min-image env sum + tiny MLP).

Algorithm (mathematically equivalent reformulation of the reference):

 1. Per-component wrapped squared displacement is periodic in dx with period
    L=10, so  wrap(dx)^2 ~= B0 + sum_n Bn cos(2*pi*n*dx/L)  (constrained
    least-squares fit, N=16 harmonics).  Hence d2[i,j] is an inner product of
    trig embeddings E[col, particle] = sin(2*pi*frac(n*x_k/L + phase)):
    one 98-deep fp32r TensorEngine matmul per 128-row block.  A constant
    embedding row adds U/V to every pair:  t' = d2 + U/V.
 2. The envelope f(t) = exp(-t)*0.5*(1+cos(pi*sqrt(t)/RC)) (t=d2) is
    approximated by  W0 + (U + V*t)*exp(-A*t)  (fit rms 5.8e-5, as good as a
    2-exp fit).  Per pair:  ONE Exp activation  er = exp(-A*t' + A*U/V)
    = exp(-A*d2), then one DVE multiply  m = t' * er = (d2 + U/V)*exp(-A*d2).
    V is folded into W1 on the host:  V*sum_j m[i,j] = sum_j (U + V*d2)e^{-A d2}.
 3. Row sums run as ones-column PE matmuls over partitions (m is symmetric,
    so column sums == row sums).  Constant + diagonal corrections are folded
    into the MLP bias:  b1' = b1 + (511*W0 - U) * W1 @ ones.
 4. MLP: h = relu(inputt @ (V*W1).T + b1'), out = sigmoid(h @ W2.T) computed
    as 1/(1 + exp(-z)) so the whole phase-2/3 ACT stream stays on the
    exp-table set (one table swap total, after the Sin phase).
 5. u = n*(x/L) + phase is computed with an exact bf16 hi/lo split of x/L
    (n and phases are bf16-exact), making the phase-1 matmul a 1-pass bf16
    matmul instead of a 4-pass fp32 one.

Sharding: pure data parallel, batch 128 -> 8 cores x 16.
"""

import numpy as np

# ---------------------------------------------------------------- constants
L = 10.0
NP = 512
BTOT = 128
NCORES = 8
BLOC = BTOT // NCORES  # 16
NH = 16                # harmonics
K = 6 * NH + 2         # 97 embedding rows + 1 const row
NUM_NODES = 256
NCH = 8                # phase-1 chunks
CW = BLOC * NP // NCH  # 1024 columns per chunk

# wrap2(theta) ~= sum_n B[n] cos(n theta)
B_HARM = [
    8.336507198660753, -10.134305777836879, 2.5283072633082164,
    -1.1207547738471013, 0.6351791173907125, -0.41237594667899846,
    0.28478810229590223, -0.20163605059415754, 0.15059719920404221,
    -0.12490354747428888, 0.11118898587488348, -0.09477489833163562,
    0.06985971056432684, -0.041620415059490684, 0.018837434788739185,
    -0.005869820105041354, 0.0009762178400180537,
]

# envelope fit: f(t) ~= W0 + (U + V*t) * exp(-A*t)
W0 = 9.80040725e-06
UF = 1.00020579e+00
VF = -2.10501370e-01
AF_ = 1.18713660e+00
UV = UF / VF                  # -4.7515...
BETA = AF_ * UF / VF          # exp bias compensation

f32 = np.float32


def _host_constants():
    import ml_dtypes
    bf16 = ml_dtypes.bfloat16
    # mt2 [7, K]: rows 0-2 = n (hi part of x/L), rows 3-5 = n (lo part),
    # row 6 = phase (0.25 turns for cos columns, 0 for sin columns).
    mt2 = np.zeros((7, K), f32)
    bcol = np.zeros((K, 1), f32)
    mt2[6, 0] = 0.25           # const col: sin(2*pi*0.25) = 1
    bcol[0, 0] = f32(UV)       # t' = d2 + U/V
    col = 1
    for k in range(3):
        for n in range(1, NH + 1):
            mt2[k, col] = n; mt2[k + 3, col] = n
            mt2[6, col] = 0.25
            bcol[col, 0] = 3.0 * B_HARM[0] / 96.0 + B_HARM[n] if False else B_HARM[n]
            col += 1
            mt2[k, col] = n; mt2[k + 3, col] = n
            mt2[6, col] = 0.0
            bcol[col, 0] = B_HARM[n]
            col += 1
    # fold the B0 constant (3*B_HARM[0] per pair) into the const column:
    # const col contributes bcol[0]*1*1 per pair -> put 3*B0 + U/V there.
    bcol[0, 0] = f32(3.0 * B_HARM[0] + UV)
    eye16 = np.eye(16, dtype=f32)
    mt2 = mt2.astype(bf16)
    return mt2, bcol, eye16


_CACHE = {}


def _build_program():
    import concourse.bacc as bacc
    import concourse.mybir as mybir
    import concourse.tile as tile

    nc = bacc.Bacc("TRN2", target_bir_lowering=False, debug=False,
                   num_devices=NCORES)
    dt = mybir.dt
    AF = mybir.ActivationFunctionType
    ALU = mybir.AluOpType
    TWO_PI = 2.0 * float(np.pi)

    xa_d = nc.declare_dram_parameter("xa", (7, BLOC * NP), dt.bfloat16, isOutput=False)
    mt_d = nc.declare_dram_parameter("mt", (7, K), dt.bfloat16, isOutput=False)
    bcol_d = nc.declare_dram_parameter("bcol", (K, 1), dt.float32, isOutput=False)
    w1t_d = nc.declare_dram_parameter("w1t", (NP, NUM_NODES), dt.float32, isOutput=False)
    b1p_d = nc.declare_dram_parameter("b1p", (1, NUM_NODES), dt.float32, isOutput=False)
    w2r_d = nc.declare_dram_parameter("w2r", (BLOC, NUM_NODES), dt.float32, isOutput=False)
    eye_d = nc.declare_dram_parameter("eye16", (16, 16), dt.float32, isOutput=False)
    y_d = nc.declare_dram_parameter("y", (BLOC, 1), dt.float32, isOutput=True)

    with tile.TileContext(nc) as tc:
        with tc.tile_pool(name="const", bufs=1) as cpool:
            xa_s = cpool.tile([7, BLOC * NP], dt.bfloat16)
            nc.gpsimd.dma_start(xa_s[:], xa_d[:])
            mt_s = cpool.tile([7, K], dt.bfloat16)
            nc.gpsimd.dma_start(mt_s[:], mt_d[:])
            bcol_s = cpool.tile([K, 1], dt.float32)
            nc.gpsimd.dma_start(bcol_s[:], bcol_d[:])
            w1t_s = cpool.tile([128, 4 * NUM_NODES], dt.float32)
            for c in range(4):
                nc.gpsimd.dma_start(
                    w1t_s[:, c * NUM_NODES:(c + 1) * NUM_NODES],
                    w1t_d[c * 128:(c + 1) * 128, :])
            b1p_s = cpool.tile([1, NUM_NODES], dt.float32)
            nc.gpsimd.dma_start(b1p_s[:], b1p_d[:])
            w2r_s = cpool.tile([BLOC, NUM_NODES], dt.float32)
            nc.gpsimd.dma_start(w2r_s[:], w2r_d[:])
            eye_s = cpool.tile([16, 16], dt.float32)
            nc.gpsimd.dma_start(eye_s[:], eye_d[:])
            ones1_s = cpool.tile([1, BLOC], dt.float32)
            nc.gpsimd.memset(ones1_s[:], 1.0)
            onesb_s = cpool.tile([128, 1], dt.bfloat16)
            nc.gpsimd.memset(onesb_s[:], 1.0)
            expb_s = cpool.tile([128, 1], dt.float32)
            nc.gpsimd.memset(expb_s[:], float(BETA))

            E_s = cpool.tile([K, BLOC * NP], dt.float32r, name="E")
            Ew_s = cpool.tile([K, BLOC * NP], dt.float32r, name="Ew")
            scopy = cpool.tile([BLOC, NP], dt.float32)

            # ---------------- phase 1: trig embeddings, 8 chunks ----------------
            with (
                tc.tile_pool(name="upsum", bufs=1, space="PSUM") as upsum,
                tc.tile_pool(name="ri", bufs=2) as ripool,
                tc.tile_pool(name="vv", bufs=2) as vpool,
            ):
                for c in range(NCH):
                    sl = slice(c * CW, (c + 1) * CW)
                    u = upsum.tile([K, CW], dt.float32, tag="u")
                    for hh in range(CW // 512):
                        nc.tensor.matmul(
                            u[:, hh * 512:(hh + 1) * 512], mt_s[:],
                            xa_s[:, c * CW + hh * 512: c * CW + (hh + 1) * 512],
                            start=True, stop=True)
                    ri = ripool.tile([K, CW], dt.int32, tag="ri")
                    nc.vector.tensor_copy(ri[:], u[:])          # round to nearest
                    v = vpool.tile([K, CW], dt.float32, tag="v")
                    nc.vector.tensor_tensor(v[:], u[:], ri[:], ALU.subtract)
                    nc.scalar.activation(E_s[:, sl], v[:], AF.Sin, scale=TWO_PI)
                    nc.scalar.activation(Ew_s[:, sl], E_s[:, sl], AF.Copy,
                                         scale=bcol_s[:, 0:1])

                # keep all Sin ops ahead of all Exp ops in the ACT stream
                tc.no_sync_barrier()

                # ---------------- phase 2: pair blocks ----------------
                with (
                    tc.tile_pool(name="tpsum", bufs=2, space="PSUM") as tpsum,
                    tc.tile_pool(name="spsum", bufs=2, space="PSUM") as spsum,
                    tc.tile_pool(name="er", bufs=2) as erpool,
                    tc.tile_pool(name="mm", bufs=2) as mpool,
                    tc.tile_pool(name="ssb", bufs=2) as ssbpool,
                ):
                    for b in range(BLOC):
                        bsl = slice(b * NP, (b + 1) * NP)
                        srow = spsum.tile([1, NP], dt.float32, tag="srow")
                        i_acc = 0
                        for g in range(2):
                            t = tpsum.tile([128, 2 * 512], dt.float32, tag="t")
                            for jj in range(2):
                                jc = 2 * g + jj
                                nc.tensor.matmul(
                                    t[:, jj * 512:(jj + 1) * 512],
                                    Ew_s[:, b * NP + jc * 128: b * NP + (jc + 1) * 128],
                                    E_s[:, bsl],
                                    start=True, stop=True)
                            er = erpool.tile([128, 2 * 512], dt.bfloat16, tag="er")
                            nc.scalar.activation(er[:], t[:], AF.Exp,
                                                 scale=-float(AF_), bias=expb_s[:])
                            m = mpool.tile([128, 2 * 512], dt.bfloat16, tag="m")
                            nc.vector.tensor_tensor(m[:], t[:], er[:], ALU.mult)
                            for jj in range(2):
                                nc.tensor.matmul(
                                    srow[:], onesb_s[:],
                                    m[:, jj * 512:(jj + 1) * 512],
                                    start=(i_acc == 0), stop=(i_acc == 3),
                                    skip_group_check=True)
                                i_acc += 1
                        ssb = ssbpool.tile([1, NP], dt.float32, tag="ssb")
                        nc.vector.tensor_copy(ssb[:], srow[:])
                        nc.gpsimd.dma_start(scopy[b:b + 1, :], ssb[:])

            # ---------------- phase 3: MLP tail ----------------
            with (
                tc.tile_pool(name="trpsum", bufs=2, space="PSUM") as trpsum,
                tc.tile_pool(name="hpsum", bufs=1, space="PSUM") as hpsum,
                tc.tile_pool(name="tail", bufs=1) as tail,
            ):
                h = hpsum.tile([BLOC, NUM_NODES], dt.float32)
                for c in range(4):
                    tp = trpsum.tile([128, BLOC], dt.float32, tag="tp")
                    nc.tensor.transpose(
                        tp[:], scopy[:, c * 128:(c + 1) * 128], eye_s[:])
                    itp = tail.tile([128, BLOC], dt.float32,
                                    tag=f"itp{c}", name=f"itp{c}")
                    nc.vector.tensor_copy(itp[:], tp[:])
                    nc.tensor.matmul(
                        h[:], itp[:],
                        w1t_s[:, c * NUM_NODES:(c + 1) * NUM_NODES],
                        start=(c == 0), stop=False,
                        skip_group_check=True)
                nc.tensor.matmul(h[:], ones1_s[:], b1p_s[:],
                                 start=False, stop=True,
                                 skip_group_check=True)
                hr = tail.tile([BLOC, NUM_NODES], dt.float32)
                nc.scalar.activation(hr[:], h[:], AF.Relu)
                hw = tail.tile([BLOC, NUM_NODES], dt.float32)
                nc.vector.tensor_tensor(hw[:], hr[:], w2r_s[:], ALU.mult)
                z = tail.tile([BLOC, 1], dt.float32)
                nc.vector.reduce_sum(z[:], hw[:], axis=mybir.AxisListType.X)
                # sigmoid(z) = 1 / (1 + exp(-z))  -- stays on the exp table set
                ez = tail.tile([BLOC, 1], dt.float32)
                nc.scalar.activation(ez[:], z[:], AF.Exp, scale=-1.0)
                ez1 = tail.tile([BLOC, 1], dt.float32)
                nc.vector.tensor_scalar(ez1[:], ez[:], 1.0, None, ALU.add)
                ys = tail.tile([BLOC, 1], dt.float32)
                nc.vector.reciprocal(ys[:], ez1[:])
                nc.gpsimd.dma_start(y_d[:], ys[:])

    nc.finalize()
    return nc


def _get_program():
    if "nc" not in _CACHE:
        _CACHE["nc"] = _build_program()
    return _CACHE["nc"]


def _make_in_maps(x, W1, b1, W2):
    import ml_dtypes
    bf16 = ml_dtypes.bfloat16
    mt2, bcol, eye16 = _host_constants()
    W1 = np.asarray(W1, f32)
    w1tv = np.ascontiguousarray((f32(VF) * W1).T).astype(f32)
    b1p = (np.asarray(b1, f32)
           + (511.0 * f32(W0) - f32(UF)) * W1.sum(axis=1)).reshape(1, NUM_NODES).astype(f32)
    w2r = np.broadcast_to(np.asarray(W2, f32).reshape(1, NUM_NODES),
                          (BLOC, NUM_NODES)).copy()
    x = np.asarray(x, f32)
    xs = (x / f32(L)).astype(f32)
    in_maps = []
    for c in range(NCORES):
        xc = xs[c * BLOC:(c + 1) * BLOC]                         # [16,512,3]
        xT = np.transpose(xc, (2, 0, 1)).reshape(3, BLOC * NP)   # [3,16*512]
        hi = xT.astype(bf16)
        lo = (xT - hi.astype(f32)).astype(bf16)
        xa = np.concatenate([hi, lo,
                             np.ones((1, BLOC * NP), bf16)], axis=0)
        in_maps.append({
            "xa": np.ascontiguousarray(xa),
            "mt": mt2, "bcol": bcol,
            "w1t": w1tv, "b1p": b1p, "w2r": w2r, "eye16": eye16,
        })
    return in_maps


def kernel(x, W1, b1, W2, _trace=False, _trace_kwargs=None):
    from concourse.bass_utils import run_bass_kernel_spmd

    nc = _get_program()
    in_maps = _make_in_maps(x, W1, b1, W2)
    res = run_bass_kernel_spmd(nc, in_maps, list(range(NCORES)),
                               trace=_trace, **(_trace_kwargs or {}))
    out = np.concatenate([res.results[c]["y"] for c in range(NCORES)], axis=0)
    if _trace:
        _CACHE["last_result"] = res
    return out.astype(f32)


# revision 5
# speedup vs baseline: 1.1989x; 1.1989x over previous
"""Trainium2 Bass kernel for CommittorNetBP (pairwise min-image env sum + tiny MLP).

Algorithm (mathematically equivalent reformulation of the reference):

 1. Per-component wrapped squared displacement is periodic in dx with period
    L=10, so  wrap(dx)^2 ~= B0 + sum_n Bn cos(2*pi*n*dx/L)  (constrained
    least-squares fit, N=16 harmonics).  Hence d2[i,j] is an inner product of
    trig embeddings E[col, particle] = sin(2*pi*frac(n*x_k/L + phase)):
    one 98-deep fp32r TensorEngine matmul per 128-row block.  A constant
    embedding row adds U/V to every pair:  t' = d2 + U/V.
 2. The envelope f(t) = exp(-t)*0.5*(1+cos(pi*sqrt(t)/RC)) (t=d2) is
    approximated by  W0 + (U + V*t)*exp(-A*t)  (fit rms 5.8e-5, as good as a
    2-exp fit).  Per pair:  ONE Exp activation  er = exp(-A*t' + A*U/V)
    = exp(-A*d2), then one DVE multiply  m = t' * er = (d2 + U/V)*exp(-A*d2).
    V is folded into W1 on the host:  V*sum_j m[i,j] = sum_j (U + V*d2)e^{-A d2}.
 3. Row sums run as ones-column PE matmuls over partitions (m is symmetric,
    so column sums == row sums).  Constant + diagonal corrections are folded
    into the MLP bias:  b1' = b1 + (511*W0 - U) * W1 @ ones.
 4. MLP: h = relu(inputt @ (V*W1).T + b1'), out = sigmoid(h @ W2.T) computed
    as 1/(1 + exp(-z)) so the whole phase-2/3 ACT stream stays on the
    exp-table set (one table swap total, after the Sin phase).
 5. u = n*(x/L) + phase is computed with an exact bf16 hi/lo split of x/L
    (n and phases are bf16-exact), making the phase-1 matmul a 1-pass bf16
    matmul instead of a 4-pass fp32 one.

Sharding: pure data parallel, batch 128 -> 8 cores x 16.
"""

import numpy as np

# ---------------------------------------------------------------- constants
L = 10.0
NP = 512
BTOT = 128
NCORES = 8
BLOC = BTOT // NCORES  # 16
NH = 16                # harmonics
K = 6 * NH + 2         # 97 embedding rows + 1 const row
NUM_NODES = 256
NCH = 8                # phase-1 chunks
CW = BLOC * NP // NCH  # 1024 columns per chunk

# wrap2(theta) ~= sum_n B[n] cos(n theta)
B_HARM = [
    8.336507198660753, -10.134305777836879, 2.5283072633082164,
    -1.1207547738471013, 0.6351791173907125, -0.41237594667899846,
    0.28478810229590223, -0.20163605059415754, 0.15059719920404221,
    -0.12490354747428888, 0.11118898587488348, -0.09477489833163562,
    0.06985971056432684, -0.041620415059490684, 0.018837434788739185,
    -0.005869820105041354, 0.0009762178400180537,
]

# envelope fit: f(t) ~= W0 + (U + V*t) * exp(-A*t)
W0 = 9.80040725e-06
UF = 1.00020579e+00
VF = -2.10501370e-01
AF_ = 1.18713660e+00
UV = UF / VF                  # -4.7515...
BETA = AF_ * UF / VF          # exp bias compensation

f32 = np.float32


def _host_constants():
    import ml_dtypes
    bf16 = ml_dtypes.bfloat16
    # mt2 [7, K]: rows 0-2 = n (hi part of x/L), rows 3-5 = n (lo part),
    # row 6 = phase (0.25 turns for cos columns, 0 for sin columns).
    mt2 = np.zeros((7, K), f32)
    bcol = np.zeros((K, 1), f32)
    mt2[6, 0] = 0.25           # const col: sin(2*pi*0.25) = 1
    bcol[0, 0] = f32(UV)       # t' = d2 + U/V
    col = 1
    for k in range(3):
        for n in range(1, NH + 1):
            mt2[k, col] = n; mt2[k + 3, col] = n
            mt2[6, col] = 0.25
            bcol[col, 0] = 3.0 * B_HARM[0] / 96.0 + B_HARM[n] if False else B_HARM[n]
            col += 1
            mt2[k, col] = n; mt2[k + 3, col] = n
            mt2[6, col] = 0.0
            bcol[col, 0] = B_HARM[n]
            col += 1
    # fold the B0 constant (3*B_HARM[0] per pair) into the const column:
    # const col contributes bcol[0]*1*1 per pair -> put 3*B0 + U/V there.
    bcol[0, 0] = f32(3.0 * B_HARM[0] + UV)
    eye16 = np.eye(16, dtype=f32)
    mt2 = mt2.astype(bf16)
    return mt2, bcol, eye16


_CACHE = {}


def _build_program():
    import concourse.bacc as bacc
    import concourse.mybir as mybir
    import concourse.tile as tile

    nc = bacc.Bacc("TRN2", target_bir_lowering=False, debug=False,
                   num_devices=NCORES)
    dt = mybir.dt
    AF = mybir.ActivationFunctionType
    ALU = mybir.AluOpType
    TWO_PI = 2.0 * float(np.pi)

    xa_d = nc.declare_dram_parameter("xa", (7, BLOC * NP), dt.bfloat16, isOutput=False)
    mt_d = nc.declare_dram_parameter("mt", (7, K), dt.bfloat16, isOutput=False)
    bcol_d = nc.declare_dram_parameter("bcol", (K, 1), dt.float32, isOutput=False)
    w1t_d = nc.declare_dram_parameter("w1t", (NP, NUM_NODES), dt.float32, isOutput=False)
    b1p_d = nc.declare_dram_parameter("b1p", (1, NUM_NODES), dt.float32, isOutput=False)
    w2r_d = nc.declare_dram_parameter("w2r", (BLOC, NUM_NODES), dt.float32, isOutput=False)
    eye_d = nc.declare_dram_parameter("eye16", (16, 16), dt.float32, isOutput=False)
    y_d = nc.declare_dram_parameter("y", (BLOC, 1), dt.float32, isOutput=True)

    with tile.TileContext(nc) as tc:
        with tc.tile_pool(name="const", bufs=1) as cpool:
            xa_s = cpool.tile([7, BLOC * NP], dt.bfloat16)
            nc.gpsimd.dma_start(xa_s[:], xa_d[:])
            mt_s = cpool.tile([7, K], dt.bfloat16)
            nc.gpsimd.dma_start(mt_s[:], mt_d[:])
            bcol_s = cpool.tile([K, 1], dt.float32)
            nc.gpsimd.dma_start(bcol_s[:], bcol_d[:])
            w1t_s = cpool.tile([128, 4 * NUM_NODES], dt.float32)
            for c in range(4):
                nc.gpsimd.dma_start(
                    w1t_s[:, c * NUM_NODES:(c + 1) * NUM_NODES],
                    w1t_d[c * 128:(c + 1) * 128, :])
            b1p_s = cpool.tile([1, NUM_NODES], dt.float32)
            nc.gpsimd.dma_start(b1p_s[:], b1p_d[:])
            w2r_s = cpool.tile([BLOC, NUM_NODES], dt.float32)
            nc.gpsimd.dma_start(w2r_s[:], w2r_d[:])
            eye_s = cpool.tile([16, 16], dt.float32)
            nc.gpsimd.dma_start(eye_s[:], eye_d[:])
            ones1_s = cpool.tile([1, BLOC], dt.float32)
            nc.gpsimd.memset(ones1_s[:], 1.0)
            onesb_s = cpool.tile([128, 1], dt.bfloat16)
            nc.gpsimd.memset(onesb_s[:], 1.0)
            expb_s = cpool.tile([128, 1], dt.float32)
            nc.gpsimd.memset(expb_s[:], float(BETA))

            E_s = cpool.tile([K, BLOC * NP], dt.float32r, name="E")
            Ew_s = cpool.tile([K, BLOC * NP], dt.float32r, name="Ew")
            scopy = cpool.tile([BLOC, NP], dt.float32)

            # -------- phases 1+2 software-pipelined over chunks/batches --------
            # phase-1 chunk c embeds batches (2c, 2c+1); phase-2 for those
            # batches is emitted one chunk later so every engine stays fed.
            # Within phase 2, batch b's srow matmuls (which depend on ACT exp
            # and DVE m) are emitted AFTER batch b+1's t-matmuls so the PE
            # never stalls behind the exp->m chain.
            with (
                tc.tile_pool(name="upsum", bufs=1, space="PSUM") as upsum,
                tc.tile_pool(name="ri", bufs=2) as ripool,
                tc.tile_pool(name="vv", bufs=2) as vpool,
                tc.tile_pool(name="tpsum", bufs=2, space="PSUM") as tpsum,
                tc.tile_pool(name="spsum", bufs=2, space="PSUM") as spsum,
                tc.tile_pool(name="er", bufs=2) as erpool,
                tc.tile_pool(name="mm", bufs=4) as mpool,
            ):
                def phase1_chunk(c):
                    sl = slice(c * CW, (c + 1) * CW)
                    u = upsum.tile([K, CW], dt.float32, tag="u")
                    for hh in range(CW // 512):
                        nc.tensor.matmul(
                            u[:, hh * 512:(hh + 1) * 512], mt_s[:],
                            xa_s[:, c * CW + hh * 512: c * CW + (hh + 1) * 512],
                            start=True, stop=True)
                    ri = ripool.tile([K, CW], dt.int32, tag="ri")
                    nc.vector.tensor_copy(ri[:], u[:])          # round to nearest
                    v = vpool.tile([K, CW], dt.float32, tag="v")
                    nc.vector.tensor_tensor(v[:], u[:], ri[:], ALU.subtract)
                    nc.scalar.activation(E_s[:, sl], v[:], AF.Sin, scale=TWO_PI)
                    nc.scalar.activation(Ew_s[:, sl], E_s[:, sl], AF.Copy,
                                         scale=bcol_s[:, 0:1])

                pend = []   # (b, srow, [m tiles]) awaiting srow emission

                def drain_pending():
                    b, srow, mts = pend.pop(0)
                    i_acc = 0
                    for m in mts:
                        for jj in range(2):
                            nc.tensor.matmul(
                                srow[:], onesb_s[:],
                                m[:, jj * 512:(jj + 1) * 512],
                                start=(i_acc == 0), stop=(i_acc == 3),
                                skip_group_check=True)
                            i_acc += 1
                    ssb = erpool.tile([1, NP], dt.float32, tag="ssb")
                    nc.vector.tensor_copy(ssb[:], srow[:])
                    nc.gpsimd.dma_start(scopy[b:b + 1, :], ssb[:])

                def phase2_batch(b):
                    bsl = slice(b * NP, (b + 1) * NP)
                    srow = spsum.tile([1, NP], dt.float32, tag="srow")
                    mts = []
                    for g in range(2):
                        t = tpsum.tile([128, 2 * 512], dt.float32, tag="t")
                        for jj in range(2):
                            jc = 2 * g + jj
                            nc.tensor.matmul(
                                t[:, jj * 512:(jj + 1) * 512],
                                Ew_s[:, b * NP + jc * 128: b * NP + (jc + 1) * 128],
                                E_s[:, bsl],
                                start=True, stop=True)
                        er = erpool.tile([128, 2 * 512], dt.bfloat16, tag="er")
                        nc.scalar.activation(er[:], t[:], AF.Exp,
                                             scale=-float(AF_), bias=expb_s[:])
                        m = mpool.tile([128, 2 * 512], dt.bfloat16, tag="m")
                        nc.vector.tensor_tensor(m[:], t[:], er[:], ALU.mult)
                        mts.append(m)
                    pend.append((b, srow, mts))
                    if len(pend) > 1:
                        drain_pending()

                # all Sin ops strictly before all Exp ops on the ACT stream
                # (sin and exp live in different ACT table sets; interleaving
                # would cost a 1283ns table load per transition).
                for c in range(NCH):
                    phase1_chunk(c)
                tc.no_sync_barrier()
                for b in range(BLOC):
                    phase2_batch(b)
                while pend:
                    drain_pending()

            # ---------------- phase 3: MLP tail ----------------
            with (
                tc.tile_pool(name="trpsum", bufs=2, space="PSUM") as trpsum,
                tc.tile_pool(name="hpsum", bufs=1, space="PSUM") as hpsum,
                tc.tile_pool(name="tail", bufs=1) as tail,
            ):
                h = hpsum.tile([BLOC, NUM_NODES], dt.float32)
                for c in range(4):
                    tp = trpsum.tile([128, BLOC], dt.float32, tag="tp")
                    nc.tensor.transpose(
                        tp[:], scopy[:, c * 128:(c + 1) * 128], eye_s[:])
                    itp = tail.tile([128, BLOC], dt.float32,
                                    tag=f"itp{c}", name=f"itp{c}")
                    nc.vector.tensor_copy(itp[:], tp[:])
                    nc.tensor.matmul(
                        h[:], itp[:],
                        w1t_s[:, c * NUM_NODES:(c + 1) * NUM_NODES],
                        start=(c == 0), stop=False,
                        skip_group_check=True)
                nc.tensor.matmul(h[:], ones1_s[:], b1p_s[:],
                                 start=False, stop=True,
                                 skip_group_check=True)
                hr = tail.tile([BLOC, NUM_NODES], dt.float32)
                nc.scalar.activation(hr[:], h[:], AF.Relu)
                hw = tail.tile([BLOC, NUM_NODES], dt.float32)
                nc.vector.tensor_tensor(hw[:], hr[:], w2r_s[:], ALU.mult)
                z = tail.tile([BLOC, 1], dt.float32)
                nc.vector.reduce_sum(z[:], hw[:], axis=mybir.AxisListType.X)
                # sigmoid(z) = 1 / (1 + exp(-z))  -- stays on the exp table set
                ez = tail.tile([BLOC, 1], dt.float32)
                nc.scalar.activation(ez[:], z[:], AF.Exp, scale=-1.0)
                ez1 = tail.tile([BLOC, 1], dt.float32)
                nc.vector.tensor_scalar(ez1[:], ez[:], 1.0, None, ALU.add)
                ys = tail.tile([BLOC, 1], dt.float32)
                nc.vector.reciprocal(ys[:], ez1[:])
                nc.gpsimd.dma_start(y_d[:], ys[:])

    nc.finalize()
    return nc


def _get_program():
    if "nc" not in _CACHE:
        _CACHE["nc"] = _build_program()
    return _CACHE["nc"]


def _make_in_maps(x, W1, b1, W2):
    import ml_dtypes
    bf16 = ml_dtypes.bfloat16
    mt2, bcol, eye16 = _host_constants()
    W1 = np.asarray(W1, f32)
    w1tv = np.ascontiguousarray((f32(VF) * W1).T).astype(f32)
    b1p = (np.asarray(b1, f32)
           + (511.0 * f32(W0) - f32(UF)) * W1.sum(axis=1)).reshape(1, NUM_NODES).astype(f32)
    w2r = np.broadcast_to(np.asarray(W2, f32).reshape(1, NUM_NODES),
                          (BLOC, NUM_NODES)).copy()
    x = np.asarray(x, f32)
    xs = (x / f32(L)).astype(f32)
    in_maps = []
    for c in range(NCORES):
        xc = xs[c * BLOC:(c + 1) * BLOC]                         # [16,512,3]
        xT = np.transpose(xc, (2, 0, 1)).reshape(3, BLOC * NP)   # [3,16*512]
        hi = xT.astype(bf16)
        lo = (xT - hi.astype(f32)).astype(bf16)
        xa = np.concatenate([hi, lo,
                             np.ones((1, BLOC * NP), bf16)], axis=0)
        in_maps.append({
            "xa": np.ascontiguousarray(xa),
            "mt": mt2, "bcol": bcol,
            "w1t": w1tv, "b1p": b1p, "w2r": w2r, "eye16": eye16,
        })
    return in_maps


def kernel(x, W1, b1, W2, _trace=False, _trace_kwargs=None):
    from concourse.bass_utils import run_bass_kernel_spmd

    nc = _get_program()
    in_maps = _make_in_maps(x, W1, b1, W2)
    res = run_bass_kernel_spmd(nc, in_maps, list(range(NCORES)),
                               trace=_trace, **(_trace_kwargs or {}))
    out = np.concatenate([res.results[c]["y"] for c in range(NCORES)], axis=0)
    if _trace:
        _CACHE["last_result"] = res
    return out.astype(f32)
